# revision 5
# baseline (speedup 1.0000x reference)
# Grouped GEMM (MoE) kernel for Trainium2, 8 NeuronCores.
#
# Sharding: tensor-parallel over out_features (column parallel). Each core
# computes ALL 4096 tokens against its own 416-column slice of every
# expert's weight. No collectives; host concatenates per-core outputs
# along the feature axis. This is perfectly load balanced across cores
# regardless of the (uneven) per-expert token counts, and the program is
# identical on every core (SPMD) -- only the weight *values* differ.
#
# Dtype: inputs are cast to bf16 on host (PE runs bf16 at 4x the fp32
# rate; tolerance 2e-2 vs ~2.5e-3 bf16 error). Output is stored bf16 and
# upcast on host, halving store traffic.
#
# Layout trick: the PE contracts over the partition dim of both operands,
# so both need in_features on partitions. The host pre-transposes x once
# (x_packed [p, k, tokens]) and pre-shuffles w to [g, p, k, cols] so every
# DMA line is contiguous. Per 128-token tile of one expert's segment:
#   psum[tok, col] += x_tile[k, tok].T @ w_tile[k, col]   (k accumulated)
#
# DMAs are split into K quarters (5 k-tiles each) so the first matmul can
# start after ~1/4 of the first expert's weights have landed instead of
# waiting for the full 2.6 MB, and so chunk/expert transitions overlap
# compute at finer grain.

import os

import numpy as np

NUM_TOKENS = 4096
IN_FEATURES = 2560
OUT_FEATURES = 3328
GROUPS = 8
N_CORES = 8
COLS = OUT_FEATURES // N_CORES  # 416
P = 128
K_TILES = IN_FEATURES // P  # 20
KQ = 4  # k-quarters per tile-load
KP = K_TILES // KQ  # 5 k-tiles per quarter
CHUNK = 512  # token chunk per x DMA

LAST_EXEC_TIME_NS = None
LAST_TRACE = None
LAST_RESULT = None

_COMPILED = {}


def _build(sizes, dt_name, out_dt_name, reps=1):
    import concourse.bass as bass
    import concourse.mybir as mybir
    import concourse.tile as tile

    dt_in = getattr(mybir.dt, dt_name)
    dt_out = getattr(mybir.dt, out_dt_name)
    f32 = mybir.dt.float32

    nc = bass.Bass()
    xt_d = nc.dram_tensor(
        "xt", [P, K_TILES * NUM_TOKENS], dt_in, kind="ExternalInput"
    )
    wt_d = nc.dram_tensor(
        "wt", [GROUPS, P, K_TILES * COLS], dt_in, kind="ExternalInput"
    )
    out_d = nc.dram_tensor("out", [NUM_TOKENS, COLS], dt_out, kind="ExternalOutput")

    offs = [0]
    for s in sizes:
        offs.append(offs[-1] + int(s))

    # contiguous-line views: [p, k, ...] with k,c (or k,t) minor on the line
    xt_v = xt_d[:, :].rearrange("p (k t) -> p k t", k=K_TILES)
    wt_v = wt_d[:, :, :].rearrange("g p (k c) -> g p k c", k=K_TILES)

    with tile.TileContext(nc) as tc:
        with (
            tc.tile_pool(name="wp", bufs=3) as wp,
            tc.tile_pool(name="xp", bufs=4) as xp,
            tc.tile_pool(name="pp", bufs=8, space="PSUM") as pp,
            tc.tile_pool(name="op", bufs=3) as op,
        ):
            def body():
                _emit_body(
                    nc, wp, xp, pp, op, sizes, offs, dt_in, dt_out, f32,
                    xt_v, wt_v, out_d,
                )

            if reps > 1:
                with tc.For_i(0, reps, 1):
                    body()
            else:
                body()

    _split_waits(nc, mybir)
    nc.finalize()
    return nc


def _emit_body(nc, wp, xp, pp, op, sizes, offs, dt_in, dt_out, f32, xt_v, wt_v, out_d):
    for g in range(GROUPS):
        seg = int(sizes[g])
        off = offs[g]
        if seg == 0:
            continue
        # one weight/x tile per k-quarter -> independent deps, early start.
        # Issue order is q-major (w_q then its x quarters) so the first
        # matmul's data gets full DMA bandwidth instead of queuing behind
        # the whole expert's weights.
        n_chunks = (seg + CHUNK - 1) // CHUNK
        wtiles = []
        xtiles = [[] for _ in range(n_chunks)]
        clens = [min(CHUNK, seg - c * CHUNK) for c in range(n_chunks)]
        for q in range(KQ):
            wt = wp.tile([P, KP, COLS], dt_in, tag=f"w{q}", name=f"w_{g}_{q}")
            nc.sync.dma_start(wt[:, :, :], wt_v[g, :, q * KP : (q + 1) * KP, :])
            wtiles.append(wt)
            for c in range(n_chunks):
                cbase = c * CHUNK
                clen = clens[c]
                xt = xp.tile([P, KP, CHUNK], dt_in, tag=f"x{q}", name=f"x_{g}_{c}_{q}")
                nc.sync.dma_start(
                    xt[:, :, :clen],
                    xt_v[:, q * KP : (q + 1) * KP, off + cbase : off + cbase + clen],
                )
                xtiles[c].append(xt)
        for c in range(n_chunks):
            cbase = c * CHUNK
            clen = clens[c]
            n_m = (clen + P - 1) // P
            for m in range(n_m):
                mt = min(P, clen - m * P)
                ps = pp.tile([P, COLS], f32, tag="ps", name=f"ps_{g}_{c}_{m}")
                for k in range(K_TILES):
                    q, r = divmod(k, KP)
                    nc.tensor.matmul(
                        ps[:mt, :],
                        xtiles[c][q][:, r, m * P : m * P + mt],
                        wtiles[q][:, r, :],
                        start=(k == 0),
                        stop=(k == K_TILES - 1),
                    )
                ob = op.tile([P, COLS], dt_out, tag="o", name=f"ob_{g}_{c}_{m}")
                nc.vector.tensor_copy(ob[:mt, :], ps[:mt, :])
                r0 = off + cbase + m * P
                # stores issue from the ACT engine so their waits (on
                # the DVE copy) never stall the input-DMA issue (SP)
                nc.scalar.dma_start(out_d[r0 : r0 + mt, :], ob[:mt, :])


def _split_waits(nc, mybir):
    """This container's walrus build allows at most ONE sync wait per
    instruction ('Too many sync wait commands' otherwise). Split any
    instruction carrying N>1 waits into N-1 same-engine NoOps (one wait
    each) followed by the original instruction with the last wait. Engine
    sequencers execute in order, so semantics are preserved."""
    counter = [0]
    for blk in nc.m.functions[0].blocks:
        insts = blk.instructions
        out = []
        changed = False
        for inst in insts:
            si = inst.sync_info
            if si is not None and len(si.on_wait) > 1:
                waits = list(si.on_wait)
                for w in waits[:-1]:
                    counter[0] += 1
                    nop = mybir.InstNoOp(name=f"I-nopw-{counter[0]}")
                    nop.engine = inst.engine
                    nop.sync_info = mybir.SyncInfo(on_wait=[w], on_update=[])
                    out.append(nop)
                inst.sync_info = mybir.SyncInfo(
                    on_wait=[waits[-1]], on_update=list(si.on_update)
                )
                changed = True
            out.append(inst)
        if changed:
            insts[:] = out


def kernel(input, weight, tokens_per_expert):
    global LAST_EXEC_TIME_NS, LAST_TRACE, LAST_RESULT
    from concourse.bass_utils import run_bass_kernel_spmd

    x = np.asarray(input, dtype=np.float32)
    w = np.asarray(weight, dtype=np.float32)
    sizes = tuple(int(s) for s in np.asarray(tokens_per_expert).reshape(-1))
    assert sum(sizes) == NUM_TOKENS and len(sizes) == GROUPS
    assert x.shape == (NUM_TOKENS, IN_FEATURES)
    assert w.shape == (GROUPS, IN_FEATURES, OUT_FEATURES)

    dt_name = os.environ.get("GG_DTYPE", "bfloat16")
    out_dt_name = os.environ.get("GG_OUT_DTYPE", "bfloat16")
    import ml_dtypes

    np_dt = {"bfloat16": ml_dtypes.bfloat16, "float32": np.float32}[dt_name]

    reps = int(os.environ.get("GG_REPS", "1"))
    key = (sizes, dt_name, out_dt_name, reps)
    if key not in _COMPILED:
        _COMPILED[key] = _build(sizes, dt_name, out_dt_name, reps)
    nc = _COMPILED[key]

    # x_packed [P, K, T]: line p holds k-major, token-minor bf16 runs
    xp_host = np.ascontiguousarray(
        x.T.reshape(K_TILES, P, NUM_TOKENS).transpose(1, 0, 2)
    ).astype(np_dt)
    in_maps = []
    for c in range(N_CORES):
        # w_packed [G, P, K, C]: line (g,p) holds k-major, col-minor runs
        wc = np.ascontiguousarray(
            w[:, :, c * COLS : (c + 1) * COLS]
            .reshape(GROUPS, K_TILES, P, COLS)
            .transpose(0, 2, 1, 3)
        ).astype(np_dt)
        in_maps.append(
            {
                "xt": xp_host.reshape(P, K_TILES * NUM_TOKENS),
                "wt": wc.reshape(GROUPS, P, K_TILES * COLS),
            }
        )

    trace = os.environ.get("GG_TRACE", "0") == "1"
    res = run_bass_kernel_spmd(nc, in_maps, list(range(N_CORES)), trace=trace)
    LAST_EXEC_TIME_NS = res.exec_time_ns
    LAST_RESULT = res
    if res.instructions_and_trace is not None:
        LAST_TRACE = res.instructions_and_trace[1]

    out = np.concatenate(
        [np.asarray(res.results[c]["out"]) for c in range(N_CORES)], axis=1
    ).astype(np.float32)
    return out


# revision 6
# speedup vs baseline: 1.1686x; 1.1686x over previous
# Grouped GEMM (MoE) kernel for Trainium2, 8 NeuronCores.
#
# Sharding: tensor-parallel over out_features (column parallel). Each core
# computes ALL 4096 tokens against its own 416-column slice of every
# expert's weight. No collectives; host concatenates per-core outputs
# along the feature axis. This is perfectly load balanced across cores
# regardless of the (uneven) per-expert token counts, and the program is
# identical on every core (SPMD) -- only the weight *values* differ.
#
# Dtype: inputs are cast to bf16 on host (PE runs bf16 at 4x the fp32
# rate; tolerance 2e-2 vs ~2.5e-3 bf16 error). Output is stored bf16 and
# upcast on host, halving store traffic.
#
# Layout trick: the PE contracts over the partition dim of both operands,
# so both need in_features on partitions. The host pre-transposes x once
# (x_packed [p, k, tokens]) and pre-shuffles w to [g, p, k, cols] so every
# DMA line is contiguous. Per 128-token tile of one expert's segment:
#   psum[tok, col] += x_tile[k, tok].T @ w_tile[k, col]   (k accumulated)
#
# DMAs are split into K quarters (5 k-tiles each) so the first matmul can
# start after ~1/4 of the first expert's weights have landed instead of
# waiting for the full 2.6 MB, and so chunk/expert transitions overlap
# compute at finer grain.

import os

import numpy as np

NUM_TOKENS = 4096
IN_FEATURES = 2560
OUT_FEATURES = 3328
GROUPS = 8
N_CORES = 8
COLS = OUT_FEATURES // N_CORES  # 416
P = 128
K_TILES = IN_FEATURES // P  # 20
KQ = 4  # k-quarters per tile-load
KP = K_TILES // KQ  # 5 k-tiles per quarter
CHUNK = 512  # token chunk per x DMA

LAST_EXEC_TIME_NS = None
LAST_TRACE = None
LAST_RESULT = None

_COMPILED = {}


def _build(sizes, dt_name, out_dt_name, reps=1):
    import concourse.bass as bass
    import concourse.mybir as mybir
    import concourse.tile as tile

    dt_in = getattr(mybir.dt, dt_name)
    dt_out = getattr(mybir.dt, out_dt_name)
    f32 = mybir.dt.float32

    nc = bass.Bass()
    xt_d = nc.dram_tensor(
        "xt", [P, K_TILES * NUM_TOKENS], dt_in, kind="ExternalInput"
    )
    wt_d = nc.dram_tensor(
        "wt", [GROUPS, P, K_TILES * COLS], dt_in, kind="ExternalInput"
    )
    out_d = nc.dram_tensor("out", [NUM_TOKENS, COLS], dt_out, kind="ExternalOutput")

    offs = [0]
    for s in sizes:
        offs.append(offs[-1] + int(s))

    # contiguous-line views: [p, k, ...] with k,c (or k,t) minor on the line
    xt_v = xt_d[:, :].rearrange("p (k t) -> p k t", k=K_TILES)
    wt_v = wt_d[:, :, :].rearrange("g p (k c) -> g p k c", k=K_TILES)

    with tile.TileContext(nc) as tc:
        with (
            tc.tile_pool(name="wp", bufs=3) as wp,
            tc.tile_pool(name="xp", bufs=4) as xp,
            tc.tile_pool(name="pp", bufs=8, space="PSUM") as pp,
            tc.tile_pool(name="op", bufs=3) as op,
        ):
            def body():
                _emit_body(
                    nc, wp, xp, pp, op, sizes, offs, dt_in, dt_out, f32,
                    xt_v, wt_v, out_d,
                )

            if reps > 1:
                with tc.For_i(0, reps, 1):
                    body()
            else:
                body()

    _split_waits(nc, mybir)
    nc.finalize()
    return nc


def _emit_body(nc, wp, xp, pp, op, sizes, offs, dt_in, dt_out, f32, xt_v, wt_v, out_d):
    for g in range(GROUPS):
        seg = int(sizes[g])
        off = offs[g]
        if seg == 0:
            continue
        # one weight/x tile per k-quarter -> independent deps, early start.
        # Issue order is q-major (w_q then its x quarters) so the first
        # matmul's data gets full DMA bandwidth instead of queuing behind
        # the whole expert's weights.
        n_chunks = (seg + CHUNK - 1) // CHUNK
        wtiles = []
        xtiles = [[] for _ in range(n_chunks)]
        clens = [min(CHUNK, seg - c * CHUNK) for c in range(n_chunks)]
        def emit_x(c, q):
            cbase = c * CHUNK
            clen = clens[c]
            xt = xp.tile([P, KP, CHUNK], dt_in, tag=f"x{q}", name=f"x_{g}_{c}_{q}")
            nc.sync.dma_start(
                xt[:, :, :clen],
                xt_v[:, q * KP : (q + 1) * KP, off + cbase : off + cbase + clen],
            )
            xtiles[c].append(xt)

        # w quarters interleaved with chunk-0 x quarters (each m-tile needs
        # ALL k-quarters, so chunk 0 must complete as early as possible);
        # remaining chunks follow chunk-major.
        for q in range(KQ):
            wt = wp.tile([P, KP, COLS], dt_in, tag=f"w{q}", name=f"w_{g}_{q}")
            nc.sync.dma_start(wt[:, :, :], wt_v[g, :, q * KP : (q + 1) * KP, :])
            wtiles.append(wt)
            emit_x(0, q)
        for c in range(1, n_chunks):
            for q in range(KQ):
                emit_x(c, q)
        for c in range(n_chunks):
            cbase = c * CHUNK
            clen = clens[c]
            n_m = (clen + P - 1) // P
            for m in range(n_m):
                mt = min(P, clen - m * P)
                ps = pp.tile([P, COLS], f32, tag="ps", name=f"ps_{g}_{c}_{m}")
                for k in range(K_TILES):
                    q, r = divmod(k, KP)
                    nc.tensor.matmul(
                        ps[:mt, :],
                        xtiles[c][q][:, r, m * P : m * P + mt],
                        wtiles[q][:, r, :],
                        start=(k == 0),
                        stop=(k == K_TILES - 1),
                    )
                ob = op.tile([P, COLS], dt_out, tag="o", name=f"ob_{g}_{c}_{m}")
                nc.vector.tensor_copy(ob[:mt, :], ps[:mt, :])
                r0 = off + cbase + m * P
                # stores issue from the ACT engine so their waits (on
                # the DVE copy) never stall the input-DMA issue (SP)
                nc.scalar.dma_start(out_d[r0 : r0 + mt, :], ob[:mt, :])


def _split_waits(nc, mybir):
    """This container's walrus build allows at most ONE sync wait per
    instruction ('Too many sync wait commands' otherwise). Split any
    instruction carrying N>1 waits into N-1 same-engine NoOps (one wait
    each) followed by the original instruction with the last wait. Engine
    sequencers execute in order, so semantics are preserved."""
    counter = [0]
    for blk in nc.m.functions[0].blocks:
        insts = blk.instructions
        out = []
        changed = False
        for inst in insts:
            si = inst.sync_info
            if si is not None and len(si.on_wait) > 1:
                waits = list(si.on_wait)
                for w in waits[:-1]:
                    counter[0] += 1
                    nop = mybir.InstNoOp(name=f"I-nopw-{counter[0]}")
                    nop.engine = inst.engine
                    nop.sync_info = mybir.SyncInfo(on_wait=[w], on_update=[])
                    out.append(nop)
                inst.sync_info = mybir.SyncInfo(
                    on_wait=[waits[-1]], on_update=list(si.on_update)
                )
                changed = True
            out.append(inst)
        if changed:
            insts[:] = out


def kernel(input, weight, tokens_per_expert):
    global LAST_EXEC_TIME_NS, LAST_TRACE, LAST_RESULT
    from concourse.bass_utils import run_bass_kernel_spmd

    x = np.asarray(input, dtype=np.float32)
    w = np.asarray(weight, dtype=np.float32)
    sizes = tuple(int(s) for s in np.asarray(tokens_per_expert).reshape(-1))
    assert sum(sizes) == NUM_TOKENS and len(sizes) == GROUPS
    assert x.shape == (NUM_TOKENS, IN_FEATURES)
    assert w.shape == (GROUPS, IN_FEATURES, OUT_FEATURES)

    dt_name = os.environ.get("GG_DTYPE", "bfloat16")
    out_dt_name = os.environ.get("GG_OUT_DTYPE", "bfloat16")
    import ml_dtypes

    np_dt = {"bfloat16": ml_dtypes.bfloat16, "float32": np.float32}[dt_name]

    reps = int(os.environ.get("GG_REPS", "1"))
    key = (sizes, dt_name, out_dt_name, reps)
    if key not in _COMPILED:
        _COMPILED[key] = _build(sizes, dt_name, out_dt_name, reps)
    nc = _COMPILED[key]

    # x_packed [P, K, T]: line p holds k-major, token-minor bf16 runs
    xp_host = np.ascontiguousarray(
        x.T.reshape(K_TILES, P, NUM_TOKENS).transpose(1, 0, 2)
    ).astype(np_dt)
    in_maps = []
    for c in range(N_CORES):
        # w_packed [G, P, K, C]: line (g,p) holds k-major, col-minor runs
        wc = np.ascontiguousarray(
            w[:, :, c * COLS : (c + 1) * COLS]
            .reshape(GROUPS, K_TILES, P, COLS)
            .transpose(0, 2, 1, 3)
        ).astype(np_dt)
        in_maps.append(
            {
                "xt": xp_host.reshape(P, K_TILES * NUM_TOKENS),
                "wt": wc.reshape(GROUPS, P, K_TILES * COLS),
            }
        )

    trace = os.environ.get("GG_TRACE", "0") == "1"
    res = run_bass_kernel_spmd(nc, in_maps, list(range(N_CORES)), trace=trace)
    LAST_EXEC_TIME_NS = res.exec_time_ns
    LAST_RESULT = res
    if res.instructions_and_trace is not None:
        LAST_TRACE = res.instructions_and_trace[1]

    out = np.concatenate(
        [np.asarray(res.results[c]["out"]) for c in range(N_CORES)], axis=1
    ).astype(np.float32)
    return out
